# revision 22
# baseline (speedup 1.0000x reference)
"""Trainium2 kernel for nn_DifferentiableBiquad.

Cascade of 4 biquad IIR filters over (B=32, L=524288), f32.

The pole radii are sigmoid(logit)*0.999 (actual inputs give r_max ~
0.71), so the cascade impulse response decays below 1e-5 of its peak
within ~30 lags. The IIR is computed as a truncated FIR via banded
block-Toeplitz matmuls on the TensorEngine, in bf16 (the 2e-2 rel-err
budget dwarfs bf16 quantization at ~2e-3):

  - x is cast to bf16 and transposed on the host into xin[r] =
    [128, 1 + L/128]: partition m holds sample m of every 128-block,
    column 0 is a zero block (row-start history). HBM runs are 8KB per
    partition, and no on-device transposes or boundary fixups are
    needed: every stationary operand is a stride-8 column view.
  - Per 128 x 1024 PSUM tile (chunk p = 1024 output samples): one
    NC1-wide tail matmul (previous-block history taps, Hb columns
    128:128+NC1) plus eight banded matmuls with stationary = blocks
    8p+dlt (X columns base+1+dlt, stride 8) against Hb[:, 0:128+NC1],
    where Hb[m, n] = h[n - m]. The dlt=3 matmul is split at column 512
    so no matmul write crosses a 2KB PSUM bank; each bank's first
    matmul carries start=True, per-element has_written bits turn later
    first touches into stores.
  - PSUM f32 -> SBUF bf16 evictions alternate between the DVE and
    Activation engines into [128, 2, 1024] stage pairs; one output DMA
    per pair on the scalar HWDGE ring writes a pair-major STAGED HBM
    layout in which each partition's two chunks are one contiguous 4KB
    run (4KB writes measured faster than 2KB, reads the opposite); the
    host unpermutes with a cheap transpose. The sync ring carries ONLY
    input (2048B aligned read packets, the fastest measured size): the
    rings are FIFO, so an evict-dependent output descriptor queued
    ahead of input would stall the whole pipeline. The stage pool depth
    (5 pairs) deliberately throttles how far output posts can run ahead,
    keeping the input stream fed first.

Batch dim (32) is sharded over 8 NeuronCores (4 rows each); rows are
independent (zero initial state == zero column 0).
"""
import math

import numpy as np

NUM_FILTERS = 4
MAX_RADIUS = 0.999
B, L = 32, 524288
N_CORES = 8
ROWS_PER_CORE = B // N_CORES
NBLK = 128                    # block size == SBUF partitions
W = 1024                      # output samples per PSUM tile partition
NBLOCKS = L // NBLK           # 4096
NGROUPS = L // (NBLK * W)     # 4 psum-tile groups per row
TAP_THR = 1e-5                # impulse-response truncation threshold


# ---------------------------------------------------------------- host math
def _coeffs_f32(log_radius, raw_angle):
    lr = np.asarray(log_radius, np.float32)
    ra = np.asarray(raw_angle, np.float32)
    radius = (np.float32(1.0) / (np.float32(1.0) + np.exp(-lr, dtype=np.float32))) * np.float32(MAX_RADIUS)
    angle = (np.float32(1.0) / (np.float32(1.0) + np.exp(-ra, dtype=np.float32))) * np.float32(math.pi)
    a1 = np.float32(-2.0) * radius * np.cos(angle, dtype=np.float32)
    a2 = radius * radius
    return a1.astype(np.float32), a2.astype(np.float32)


def _impulse_response(a1, a2, b0, b1, b2, T=256):
    h = np.zeros(T, np.float64)
    h[0] = 1.0
    for f in range(NUM_FILTERS):
        s1 = s2 = 0.0
        out = np.zeros(T, np.float64)
        for n in range(T):
            xn = h[n]
            yn = float(b0[f]) * xn + s1
            s1 = float(b1[f]) * xn - float(a1[f]) * yn + s2
            s2 = float(b2[f]) * xn - float(a2[f]) * yn
            out[n] = yn
        h = out
    return h


def _build_hb(inputs):
    a1, a2 = _coeffs_f32(inputs["log_radius"], inputs["raw_angle"])
    h = _impulse_response(
        a1, a2,
        np.asarray(inputs["b0"], np.float64),
        np.asarray(inputs["b1"], np.float64),
        np.asarray(inputs["b2"], np.float64),
    )
    hmax = np.abs(h).max()
    tap_max = int(np.max(np.nonzero(np.abs(h) > TAP_THR * hmax)))
    assert tap_max <= 127, (
        f"impulse response too long for single-shift kernel (tap_max={tap_max})"
    )
    NC1 = max(1, min(128, tap_max))
    n_idx = np.arange(NBLK)
    m_idx = np.arange(NBLK)
    lag0 = n_idx[None, :] - m_idx[:, None]           # [m, n]
    H0T = np.where((lag0 >= 0) & (lag0 <= tap_max), h[np.clip(lag0, 0, 255)], 0.0)
    lag1 = 128 + n_idx[None, :NC1] - m_idx[:, None]  # [m, n]
    H1T = np.where((lag1 >= 1) & (lag1 <= tap_max), h[np.clip(lag1, 0, 255)], 0.0)
    return np.concatenate([H0T, H1T], axis=1)        # [128, 128+NC1]


# ---------------------------------------------------------------- program
_PROGRAM_CACHE = {}


def build_program(n_rows, length, NC1):
    import concourse.mybir as mybir
    from concourse import bacc
    from concourse.tile import TileContext

    f32 = mybir.dt.float32
    bf16 = mybir.dt.bfloat16
    ncols = length // NBLK + 1           # zero col + one col per block
    ngroups = length // (NBLK * W)       # psum tiles per row
    gcols = W // NBLK                    # 8 blocks per chunk
    pad = gcols - 1                      # stride-8 view bound slack

    nc = bacc.Bacc("TRN2", target_bir_lowering=False, debug=False,
                   enable_asserts=False, num_devices=N_CORES)
    xin = nc.dram_tensor("xin", [n_rows, NBLK, ncols - 1], bf16, kind="ExternalInput")
    hb = nc.dram_tensor("hb", [NBLK, NBLK + NC1], bf16, kind="ExternalInput")
    yout = nc.dram_tensor("yout", [n_rows, length], bf16, kind="ExternalOutput")

    # Staged quad-major HBM layout: partition p's four chunks of a row
    # are one contiguous 8KB run (bigger write runs drain measurably
    # faster: 2KB runs ~90 B/ns aggregate, 4KB ~380, 8KB ~420) and a
    # quarter of the descriptors. The host unpermutes with a cheap
    # transpose. yout_s[r] has dims [p, G, c] matching a [128, 4, W]
    # stage quad for the row's four groups.
    yout_s = yout.ap().rearrange("r (p G c) -> r p G c", p=NBLK, G=ngroups, c=W)

    with TileContext(nc) as tc:
        with (
            tc.tile_pool(name="const", bufs=1) as cpool,
            tc.tile_pool(name="xrow", bufs=4) as xpool,
            tc.tile_pool(name="stage", bufs=4) as spool,
            tc.tile_pool(name="py", bufs=4, space="PSUM") as pypool,
        ):
            hb_sb = cpool.tile([NBLK, NBLK + NC1], bf16, tag="hb")
            nc.scalar.dma_start(out=hb_sb[:], in_=hb.ap())

            # PE warm-up: the HAM clock gate keeps the PE at 1.2 GHz
            # until it sees ~3.4us of sustained matmul activity. The
            # real matmul stream only starts ~11us in (first input
            # chunk + semaphore latency), so without this the whole
            # kernel runs at the cold clock (measured: every real
            # matmul at the 0.83 ns/col cold rate). A burst of wide
            # dummy matmuls on a zeroed scratch tile, issued as soon as
            # the engines enter the program (~6us), has the PE warm
            # right as the first real group lands. Results go to a
            # pool PSUM tile that real groups later reclaim via
            # start=True overwrite.
            scratch = cpool.tile([NBLK, 512], bf16, tag="wm")
            nc.vector.memset(scratch[:], 0.0)
            pywarm = pypool.tile([NBLK, W], f32, tag="py")
            for _ in range(9):
                nc.tensor.matmul(
                    pywarm[:, 0:512], scratch[:, 0:NBLK], scratch[:],
                    start=True, stop=True, skip_group_check=True,
                )

            # All input DMAs up front (all rows resident) on the sync
            # ring, which carries ONLY input: output descriptors behind
            # 4MB of queued input would stall the whole pipeline (ring
            # is FIFO), and splitting input across two rings measured
            # SLOWER (two interleaved descriptor streams drained at
            # ~230 B/ns combined vs ~300 for one sequential stream).
            # Posts stay in row order — the PE consumes rows in order.
            # Zero history column via memset (on the otherwise-idle
            # DVE) so every HBM run is aligned.
            xtiles = []
            for r in range(n_rows):
                X = xpool.tile([NBLK, ncols + pad], bf16, tag="x")
                nc.vector.memset(X[:, 0:1], 0.0)
                nc.sync.dma_start(
                    out=X[:, 1:ncols],
                    in_=xin.ap()[r],
                )
                xtiles.append(X)

            for r in range(n_rows):
                X = xtiles[r]
                for g in range(ngroups):
                    base = g * W

                    def stat(col0):
                        # [128, 128] stationary: X columns col0 + 8*p
                        return X[:, col0:col0 + W].rearrange(
                            "m (c e) -> m c e", e=gcols
                        )[:, :, 0]

                    py = pypool.tile([NBLK, W], f32, tag="py")
                    # Tail: previous-block history taps into [0, NC1).
                    nc.tensor.matmul(
                        py[:, 0:NC1], stat(base),
                        hb_sb[:, NBLK:NBLK + NC1],
                        start=True, stop=False, skip_group_check=True,
                    )
                    for dlt in range(gcols):
                        lo = dlt * NBLK
                        hi = min(W, lo + NBLK + NC1)
                        st = stat(base + 1 + dlt)
                        if lo < 512 and hi > 512:
                            # Split at the PSUM bank boundary; the upper
                            # piece is bank 1's first write.
                            nc.tensor.matmul(
                                py[:, lo:512], st, hb_sb[:, 0:512 - lo],
                                start=False, stop=True, skip_group_check=True,
                            )
                            nc.tensor.matmul(
                                py[:, 512:hi], st, hb_sb[:, 512 - lo:hi - lo],
                                start=True, stop=False, skip_group_check=True,
                            )
                        else:
                            nc.tensor.matmul(
                                py[:, lo:hi], st, hb_sb[:, 0:hi - lo],
                                start=False, stop=(dlt == gcols - 1),
                                skip_group_check=True,
                            )

                    # Alternate whole-group evicts between the two
                    # PSUM-capable engines; pair two groups per output DMA
                    # (fewer posts -> less DMA-sem-pool cross-blocking).
                    # All output posts on the scalar ring: the sync ring
                    # carries only input, so no FIFO ever has an output
                    # descriptor queued ahead of input (tried routing half
                    # the posts to gpsimd: they sat behind ~2MB of queued
                    # input descriptors and the output drained in one
                    # burst after the input finished — 2us slower). The
                    # last pair posts per-group (2KB runs) so the final
                    # drain is half-sized.
                    # Each group's PSUM eviction is split in half across
                    # the two PSUM-capable engines in parallel (DVE takes
                    # bank 0, scalar bank 1): the PSUM tile frees in
                    # ~0.6us instead of ~1.2, which keeps the PE from
                    # stalling on pool recycling and keeps evicts ahead
                    # of the output drain at the end of the stream.
                    # Whole-group evicts alternating between the two
                    # PSUM-capable engines: one instruction + one
                    # cross-engine semaphore per group instead of two.
                    # (The teardown EVENT_SEMAPHORE storm at the end of
                    # the NEFF scales with the program's semaphore count
                    # — it is ~8us of the measured exec window — so
                    # fewer sync edges beat the ~0.5us of extra PSUM
                    # tile-free latency, now that the warm PE keeps the
                    # pipeline comfortably ahead of the output drain.)
                    if g == 0:
                        squad = spool.tile(
                            [NBLK, ngroups, W], bf16, tag="stage"
                        )
                    if g % 2 == 0:
                        nc.vector.tensor_copy(out=squad[:, g], in_=py[:])
                    else:
                        nc.scalar.copy(out=squad[:, g], in_=py[:])
                    # One 8KB-run quad post per row, except the LAST row
                    # posts its two pairs separately (4KB runs): the last
                    # row's output is the serial tail (compute -> evict ->
                    # post -> drain), so halving its post granularity
                    # trims ~1.5us off the end of the stream.
                    if r == n_rows - 1:
                        if g % 2 == 1:
                            nc.scalar.dma_start(
                                out=yout_s[r][:, g - 1:g + 1],
                                in_=squad[:, g - 1:g + 1],
                            )
                    elif g == ngroups - 1:
                        nc.scalar.dma_start(
                            out=yout_s[r], in_=squad[:]
                        )
    nc.compile()
    return nc


def _get_program(n_rows, length, NC1):
    key = (n_rows, length, NC1)
    if key not in _PROGRAM_CACHE:
        _PROGRAM_CACHE[key] = build_program(*key)
    return _PROGRAM_CACHE[key]


# ---------------------------------------------------------------- entry
def _run(inputs, trace=False):
    import ml_dtypes
    from concourse.bass_utils import run_bass_kernel_spmd

    bf16 = ml_dtypes.bfloat16
    x = np.asarray(inputs["x"], np.float32)
    assert x.shape == (B, L)
    Hb = _build_hb(inputs).astype(bf16)
    NC1 = Hb.shape[1] - NBLK

    # Host-side shard layout: bf16, per-row transpose to [128, nblocks].
    xt = np.ascontiguousarray(
        x.astype(bf16).reshape(B, NBLOCKS, NBLK).swapaxes(1, 2)
    )

    nc = _get_program(ROWS_PER_CORE, L, NC1)
    xs = xt.reshape(N_CORES, ROWS_PER_CORE, NBLK, NBLOCKS)
    in_maps = [{"xin": xs[c], "hb": Hb} for c in range(N_CORES)]
    res = run_bass_kernel_spmd(nc, in_maps, core_ids=list(range(N_CORES)),
                               trace=trace)
    # Undo the device's quad-major staged output layout:
    # staged[r, p, G, c] -> natural[r, G, p, c].
    ys = np.stack([np.asarray(res.results[c]["yout"]) for c in range(N_CORES)])
    ys = ys.reshape(N_CORES, ROWS_PER_CORE, NBLK, NGROUPS, W)
    y = ys.transpose(0, 1, 3, 2, 4).astype(np.float32).reshape(B, L)
    return y, res


def kernel(x, log_radius, raw_angle, b0, b1, b2):
    y, _ = _run(dict(x=x, log_radius=log_radius, raw_angle=raw_angle,
                     b0=b0, b1=b1, b2=b2))
    return y



# revision 23
# speedup vs baseline: 1.2803x; 1.2803x over previous
"""Trainium2 kernel for nn_DifferentiableBiquad.

Cascade of 4 biquad IIR filters over (B=32, L=524288), f32.

The pole radii are sigmoid(logit)*0.999 (actual inputs give r_max ~
0.71), so the cascade impulse response decays below 1e-5 of its peak
within ~30 lags. The IIR is computed as a truncated FIR via banded
block-Toeplitz matmuls on the TensorEngine, in bf16 (the 2e-2 rel-err
budget dwarfs bf16 quantization at ~2e-3):

  - x is cast to bf16 and transposed on the host into xin[r] =
    [128, 1 + L/128]: partition m holds sample m of every 128-block,
    column 0 is a zero block (row-start history). HBM runs are 8KB per
    partition, and no on-device transposes or boundary fixups are
    needed: every stationary operand is a stride-8 column view.
  - Per 128 x 1024 PSUM tile (chunk p = 1024 output samples): one
    NC1-wide tail matmul (previous-block history taps, Hb columns
    128:128+NC1) plus eight banded matmuls with stationary = blocks
    8p+dlt (X columns base+1+dlt, stride 8) against Hb[:, 0:128+NC1],
    where Hb[m, n] = h[n - m]. The dlt=3 matmul is split at column 512
    so no matmul write crosses a 2KB PSUM bank; each bank's first
    matmul carries start=True, per-element has_written bits turn later
    first touches into stores.
  - PSUM f32 -> SBUF bf16 evictions alternate between the DVE and
    Activation engines into [128, 2, 1024] stage pairs; one output DMA
    per pair on the scalar HWDGE ring writes a pair-major STAGED HBM
    layout in which each partition's two chunks are one contiguous 4KB
    run (4KB writes measured faster than 2KB, reads the opposite); the
    host unpermutes with a cheap transpose. The sync ring carries ONLY
    input (2048B aligned read packets, the fastest measured size): the
    rings are FIFO, so an evict-dependent output descriptor queued
    ahead of input would stall the whole pipeline. The stage pool depth
    (5 pairs) deliberately throttles how far output posts can run ahead,
    keeping the input stream fed first.

Batch dim (32) is sharded over 8 NeuronCores (4 rows each); rows are
independent (zero initial state == zero column 0).
"""
import math

import numpy as np

NUM_FILTERS = 4
MAX_RADIUS = 0.999
B, L = 32, 524288
N_CORES = 8
ROWS_PER_CORE = B // N_CORES
NBLK = 128                    # block size == SBUF partitions
W = 1024                      # output samples per PSUM tile partition
NBLOCKS = L // NBLK           # 4096
NGROUPS = L // (NBLK * W)     # 4 psum-tile groups per row
TAP_THR = 1e-5                # impulse-response truncation threshold


# ---------------------------------------------------------------- host math
def _coeffs_f32(log_radius, raw_angle):
    lr = np.asarray(log_radius, np.float32)
    ra = np.asarray(raw_angle, np.float32)
    radius = (np.float32(1.0) / (np.float32(1.0) + np.exp(-lr, dtype=np.float32))) * np.float32(MAX_RADIUS)
    angle = (np.float32(1.0) / (np.float32(1.0) + np.exp(-ra, dtype=np.float32))) * np.float32(math.pi)
    a1 = np.float32(-2.0) * radius * np.cos(angle, dtype=np.float32)
    a2 = radius * radius
    return a1.astype(np.float32), a2.astype(np.float32)


def _impulse_response(a1, a2, b0, b1, b2, T=256):
    h = np.zeros(T, np.float64)
    h[0] = 1.0
    for f in range(NUM_FILTERS):
        s1 = s2 = 0.0
        out = np.zeros(T, np.float64)
        for n in range(T):
            xn = h[n]
            yn = float(b0[f]) * xn + s1
            s1 = float(b1[f]) * xn - float(a1[f]) * yn + s2
            s2 = float(b2[f]) * xn - float(a2[f]) * yn
            out[n] = yn
        h = out
    return h


def _build_hb(inputs):
    a1, a2 = _coeffs_f32(inputs["log_radius"], inputs["raw_angle"])
    h = _impulse_response(
        a1, a2,
        np.asarray(inputs["b0"], np.float64),
        np.asarray(inputs["b1"], np.float64),
        np.asarray(inputs["b2"], np.float64),
    )
    hmax = np.abs(h).max()
    tap_max = int(np.max(np.nonzero(np.abs(h) > TAP_THR * hmax)))
    assert tap_max <= 127, (
        f"impulse response too long for single-shift kernel (tap_max={tap_max})"
    )
    NC1 = max(1, min(128, tap_max))
    n_idx = np.arange(NBLK)
    m_idx = np.arange(NBLK)
    lag0 = n_idx[None, :] - m_idx[:, None]           # [m, n]
    H0T = np.where((lag0 >= 0) & (lag0 <= tap_max), h[np.clip(lag0, 0, 255)], 0.0)
    lag1 = 128 + n_idx[None, :NC1] - m_idx[:, None]  # [m, n]
    H1T = np.where((lag1 >= 1) & (lag1 <= tap_max), h[np.clip(lag1, 0, 255)], 0.0)
    return np.concatenate([H0T, H1T], axis=1)        # [128, 128+NC1]


# ---------------------------------------------------------------- program
_PROGRAM_CACHE = {}


def build_program(n_rows, length, NC1):
    import concourse.mybir as mybir
    from concourse import bacc
    from concourse.tile import TileContext

    f32 = mybir.dt.float32
    bf16 = mybir.dt.bfloat16
    ncols = length // NBLK + 1           # zero col + one col per block
    ngroups = length // (NBLK * W)       # psum tiles per row
    gcols = W // NBLK                    # 8 blocks per chunk
    pad = gcols - 1                      # stride-8 view bound slack

    nc = bacc.Bacc("TRN2", target_bir_lowering=False, debug=False,
                   enable_asserts=False, num_devices=N_CORES)
    xin = nc.dram_tensor("xin", [n_rows, NBLK, ncols - 1], bf16, kind="ExternalInput")
    hb = nc.dram_tensor("hb", [NBLK, NBLK + NC1], bf16, kind="ExternalInput")
    yout = nc.dram_tensor("yout", [n_rows, length], bf16, kind="ExternalOutput")

    # Staged quad-major HBM layout: partition p's four chunks of a row
    # are one contiguous 8KB run (bigger write runs drain measurably
    # faster: 2KB runs ~90 B/ns aggregate, 4KB ~380, 8KB ~420) and a
    # quarter of the descriptors. The host unpermutes with a cheap
    # transpose. yout_s[r] has dims [p, G, c] matching a [128, 4, W]
    # stage quad for the row's four groups.
    yout_s = yout.ap().rearrange("r (p G c) -> r p G c", p=NBLK, G=ngroups, c=W)

    with TileContext(nc) as tc:
        with (
            tc.tile_pool(name="const", bufs=1) as cpool,
            tc.tile_pool(name="xrow", bufs=4) as xpool,
            tc.tile_pool(name="stage", bufs=4) as spool,
            tc.tile_pool(name="py", bufs=4, space="PSUM") as pypool,
        ):
            hb_sb = cpool.tile([NBLK, NBLK + NC1], bf16, tag="hb")
            nc.scalar.dma_start(out=hb_sb[:], in_=hb.ap())

            # PE warm-up: the HAM clock gate keeps the PE at 1.2 GHz
            # until it sees ~3.4us of sustained matmul activity. The
            # real matmul stream only starts ~11us in (first input
            # chunk + semaphore latency), so without this the whole
            # kernel runs at the cold clock (measured: every real
            # matmul at the 0.83 ns/col cold rate). A burst of wide
            # dummy matmuls on a zeroed scratch tile, issued as soon as
            # the engines enter the program (~6us), has the PE warm
            # right as the first real group lands. Results go to a
            # pool PSUM tile that real groups later reclaim via
            # start=True overwrite.
            scratch = cpool.tile([NBLK, 512], bf16, tag="wm")
            nc.vector.memset(scratch[:], 0.0)
            pywarm = pypool.tile([NBLK, W], f32, tag="py")
            for _ in range(9):
                nc.tensor.matmul(
                    pywarm[:, 0:512], scratch[:, 0:NBLK], scratch[:],
                    start=True, stop=True, skip_group_check=True,
                )

            # All input DMAs up front (all rows resident) on the sync
            # ring, which carries ONLY input: output descriptors behind
            # 4MB of queued input would stall the whole pipeline (ring
            # is FIFO), and splitting input across two rings measured
            # SLOWER (two interleaved descriptor streams drained at
            # ~230 B/ns combined vs ~300 for one sequential stream).
            # Posts stay in row order — the PE consumes rows in order.
            # Zero history column via memset (on the otherwise-idle
            # DVE) so every HBM run is aligned.
            xtiles = []
            for r in range(n_rows):
                X = xpool.tile([NBLK, ncols + pad], bf16, tag="x")
                nc.vector.memset(X[:, 0:1], 0.0)
                nc.sync.dma_start(
                    out=X[:, 1:ncols],
                    in_=xin.ap()[r],
                )
                xtiles.append(X)

            for r in range(n_rows):
                X = xtiles[r]
                for g in range(ngroups):
                    base = g * W

                    def stat(col0):
                        # [128, 128] stationary: X columns col0 + 8*p
                        return X[:, col0:col0 + W].rearrange(
                            "m (c e) -> m c e", e=gcols
                        )[:, :, 0]

                    py = pypool.tile([NBLK, W], f32, tag="py")
                    # Tail: previous-block history taps into [0, NC1).
                    nc.tensor.matmul(
                        py[:, 0:NC1], stat(base),
                        hb_sb[:, NBLK:NBLK + NC1],
                        start=True, stop=False, skip_group_check=True,
                    )
                    for dlt in range(gcols):
                        lo = dlt * NBLK
                        hi = min(W, lo + NBLK + NC1)
                        st = stat(base + 1 + dlt)
                        if lo < 512 and hi > 512:
                            # Split at the PSUM bank boundary; the upper
                            # piece is bank 1's first write.
                            nc.tensor.matmul(
                                py[:, lo:512], st, hb_sb[:, 0:512 - lo],
                                start=False, stop=True, skip_group_check=True,
                            )
                            nc.tensor.matmul(
                                py[:, 512:hi], st, hb_sb[:, 512 - lo:hi - lo],
                                start=True, stop=False, skip_group_check=True,
                            )
                        else:
                            nc.tensor.matmul(
                                py[:, lo:hi], st, hb_sb[:, 0:hi - lo],
                                start=False, stop=(dlt == gcols - 1),
                                skip_group_check=True,
                            )

                    # Alternate whole-group evicts between the two
                    # PSUM-capable engines; pair two groups per output DMA
                    # (fewer posts -> less DMA-sem-pool cross-blocking).
                    # All output posts on the scalar ring: the sync ring
                    # carries only input, so no FIFO ever has an output
                    # descriptor queued ahead of input (tried routing half
                    # the posts to gpsimd: they sat behind ~2MB of queued
                    # input descriptors and the output drained in one
                    # burst after the input finished — 2us slower). The
                    # last pair posts per-group (2KB runs) so the final
                    # drain is half-sized.
                    # Each group's PSUM eviction is split in half across
                    # the two PSUM-capable engines in parallel (DVE takes
                    # bank 0, scalar bank 1): the PSUM tile frees in
                    # ~0.6us instead of ~1.2, which keeps the PE from
                    # stalling on pool recycling and keeps evicts ahead
                    # of the output drain at the end of the stream.
                    # Each group's PSUM eviction is split in half across
                    # the two PSUM-capable engines in parallel (DVE takes
                    # bank 0, scalar bank 1): the PSUM tile frees in
                    # ~0.7us and both engines stay locked to the same
                    # group, which keeps the pipeline cadence tight.
                    # (Tried whole-group evicts alternating engines to
                    # halve the semaphore count: the 1.5us strided copies
                    # de-synchronized the chain and cost 9us.)
                    if g == 0:
                        squad = spool.tile(
                            [NBLK, ngroups, W], bf16, tag="stage"
                        )
                    nc.vector.tensor_copy(
                        out=squad[:, g, 0:512], in_=py[:, 0:512]
                    )
                    nc.scalar.copy(
                        out=squad[:, g, 512:W], in_=py[:, 512:W]
                    )
                    # One 8KB-run quad post per row, except the LAST row
                    # posts its two pairs separately (4KB runs): the last
                    # row's output is the serial tail (compute -> evict ->
                    # post -> drain), so halving its post granularity
                    # trims ~1.5us off the end of the stream.
                    if r == n_rows - 1:
                        if g % 2 == 1:
                            nc.scalar.dma_start(
                                out=yout_s[r][:, g - 1:g + 1],
                                in_=squad[:, g - 1:g + 1],
                            )
                    elif g == ngroups - 1:
                        nc.scalar.dma_start(
                            out=yout_s[r], in_=squad[:]
                        )
    nc.compile()
    return nc


def _get_program(n_rows, length, NC1):
    key = (n_rows, length, NC1)
    if key not in _PROGRAM_CACHE:
        _PROGRAM_CACHE[key] = build_program(*key)
    return _PROGRAM_CACHE[key]


# ---------------------------------------------------------------- entry
def _run(inputs, trace=False):
    import ml_dtypes
    from concourse.bass_utils import run_bass_kernel_spmd

    bf16 = ml_dtypes.bfloat16
    x = np.asarray(inputs["x"], np.float32)
    assert x.shape == (B, L)
    Hb = _build_hb(inputs).astype(bf16)
    NC1 = Hb.shape[1] - NBLK

    # Host-side shard layout: bf16, per-row transpose to [128, nblocks].
    xt = np.ascontiguousarray(
        x.astype(bf16).reshape(B, NBLOCKS, NBLK).swapaxes(1, 2)
    )

    nc = _get_program(ROWS_PER_CORE, L, NC1)
    xs = xt.reshape(N_CORES, ROWS_PER_CORE, NBLK, NBLOCKS)
    in_maps = [{"xin": xs[c], "hb": Hb} for c in range(N_CORES)]
    res = run_bass_kernel_spmd(nc, in_maps, core_ids=list(range(N_CORES)),
                               trace=trace)
    # Undo the device's quad-major staged output layout:
    # staged[r, p, G, c] -> natural[r, G, p, c].
    ys = np.stack([np.asarray(res.results[c]["yout"]) for c in range(N_CORES)])
    ys = ys.reshape(N_CORES, ROWS_PER_CORE, NBLK, NGROUPS, W)
    y = ys.transpose(0, 1, 3, 2, 4).astype(np.float32).reshape(B, L)
    return y, res


def kernel(x, log_radius, raw_angle, b0, b1, b2):
    y, _ = _run(dict(x=x, log_radius=log_radius, raw_angle=raw_angle,
                     b0=b0, b1=b1, b2=b2))
    return y

